# revision 1
# baseline (speedup 1.0000x reference)
"""Trainium2 Bass kernel for nn_DetectionLoss (YOLOv8-style detection loss), v2.

SPMD: 2 images per core, 8 cores. Host precomputes GT-derived geometry
(rect windows, in-box masks, GT tables) and pre-gathers rect candidate
boxes/scores (pure data movement, like the baseline's transposed-cls
upload); all math runs on device.

Activation eras to avoid ACT_TABLE_LOAD thrash:
  era B (sigmoid_and_others): stream sig=Sigmoid(-x), rect score sigmoid,
  all arctans.  era A (natural_log_exp_and_others): stream L=Ln(sig), DFL
  log-softmax, per-slot sqrt/softplus via exp/ln.  Era A acts are gated on
  a zero-column that depends on the last era-B outputs.
bg term: 0.75*p^2*softplus(x) = -0.75*(1-sig)^2*L with sig=sigmoid(-x).
"""
import sys
import numpy as np
import ml_dtypes
from contextlib import ExitStack

try:
    import concourse.bass  # noqa: F401
except ImportError:
    for _p in ("/opt/trn_rl_repo", "/root/.axon_site/_ro/trn_rl_repo"):
        if _p not in sys.path:
            sys.path.append(_p)

import concourse.bass as bass
import concourse.mybir as mybir
import concourse.tile as tile
from concourse import bacc
from concourse.bass import AP
from concourse.bass_utils import run_bass_kernel_spmd

dt = mybir.dt
Alu = mybir.AluOpType
Act = mybir.ActivationFunctionType
f32 = dt.float32
bf16 = dt.bfloat16

NCORES = 8
BPC = 2
N = 33600
C = 80
M = 32
K10 = 10
EPS = 1e-9
EPS2 = 1e-18
CEPS = 1e-7
NEG = -1e30
LVLS = [(0, 160, 8, 42, 21), (25600, 80, 16, 22, 11), (32000, 40, 32, 12, 6)]
SEG = [Rh * R for (_, _, _, R, Rh) in LVLS]
SEGOFF = [0, SEG[0], SEG[0] + SEG[1]]
NC1 = sum(SEG)                                          # 1196
NTILE = 6
STREAM_F = 2800
NT = (N * C * BPC) // (128 * STREAM_F)                  # 30
P = 128


def _patch_act_tables():
    """Resolve exp/ln to natural_log_exp_and_others in the ATL placement
    pass (instead of exp_and_others / natural_log) so exp+ln sequences
    share one table set. Set indices are unchanged."""
    import concourse.hw_specs as hw_specs
    if getattr(bacc, "_act_tabs_patched", False):
        return
    orig = hw_specs.get_activation_tables

    def patched(arch):
        out = {}
        for name, funcs in orig(arch).items():
            funcs = set(funcs)
            if name == "exp_and_others":
                funcs.discard(Act.Exp)
            if name == "natural_log":
                funcs.discard(Act.Ln)
            out[name] = funcs
        return out

    bacc.get_activation_tables = patched
    bacc._act_tabs_patched = True


# ===================== host precompute =====================

def _host_consts():
    p = np.arange(P)
    iotam192 = np.broadcast_to(np.tile(np.arange(M, dtype=np.float32), 6),
                               (P, 6 * M)).copy()
    i16 = np.broadcast_to(np.tile(np.arange(16, dtype=np.float32), 24),
                          (P, 384)).copy()
    i80 = np.broadcast_to(np.tile(np.arange(80, dtype=np.float32), NTILE),
                          (P, 480)).copy()
    t6 = np.arange(NTILE)[None, :]
    sidx = p[:, None] * 3 + (t6 % 3)
    slotneg = -(1.0 + (t6 // 3) * 384 + sidx).astype(np.float32)
    mk = (sidx // 12).astype(np.float32)
    img6 = (t6 // 3).astype(np.float32) * np.ones((P, 1), np.float32)
    ident = np.eye(P, dtype=np.float32)
    iota12 = np.broadcast_to(np.arange(12, dtype=np.float32), (64, 12)).copy()
    ones1 = np.ones((P, 1), ml_dtypes.bfloat16)
    return {
        "hc_iotam": iotam192, "hc_i16": i16, "hc_i80": i80,
        "hc_slotneg": slotneg, "hc_mk": mk, "hc_img6": img6,
        "hc_ident": ident, "hc_iota12": iota12, "hc_ones1": ones1,
    }


def _host_prep(cls_logits, box_decoded, targets, anchors):
    B = cls_logits.shape[0]
    f = np.float32
    lab_all = np.maximum(targets[..., 0], 0.0).astype(f)
    bx1, by1, bx2, by2 = (targets[..., i].astype(f) for i in range(1, 5))
    gw = bx2 - bx1
    gh = by2 - by1
    area = np.maximum(gw, 0) * np.maximum(gh, 0)
    gmask = ((targets[..., 0] >= 0) & (area > 0)).astype(f)
    gatan = np.arctan(gw / (gh + CEPS)).astype(f)

    i0s, j0s = [], []
    for (off, W, s, R, Rh) in LVLS:
        i0 = np.trunc(np.clip(bx1 / s - 0.5, 0.0, W - R) - 0.4999)
        j0 = np.trunc(np.clip(by1 / s - 0.5, 0.0, W - R) - 0.4999)
        i0s.append(i0.astype(np.int64))
        j0s.append(j0.astype(np.int64))

    aidx = np.zeros((B, M, 2, NC1), np.int64)
    for li, (off, W, s, R, Rh) in enumerate(LVLS):
        r = np.arange(Rh)
        c = np.arange(R)
        for h in (0, 1):
            rows = j0s[li][..., None] + h * Rh + r              # (B,M,Rh)
            base = off + rows * W + i0s[li][..., None]
            idx = base[..., :, None] + c[None, None, None, :]   # (B,M,Rh,R)
            aidx[:, :, h, SEGOFF[li]:SEGOFF[li] + SEG[li]] = \
                idx.reshape(B, M, SEG[li])

    ax = anchors[aidx, 0].astype(f)
    ay = anchors[aidx, 1].astype(f)
    lt = np.minimum(ax - bx1[..., None, None], ay - by1[..., None, None])
    rb = np.minimum(bx2[..., None, None] - ax, by2[..., None, None] - ay)
    maskc = ((np.minimum(lt, rb) > EPS)
             & gmask[..., None, None].astype(bool)).astype(f)

    bix = np.arange(B)[:, None, None, None]
    rbox = box_decoded[bix, aidx]                               # (B,M,2,NC1,4)
    slog = cls_logits[bix, aidx, lab_all.astype(np.int64)[..., None, None]]
    # fold candidate mask into the score logit: sigmoid(-60)*ov^12 < EPS^2
    slog = np.where(maskc > 0, slog, -60.0).astype(f)

    hcs = _host_consts()
    maps = []
    for cidx in range(NCORES):
        bs = [cidx * BPC, cidx * BPC + 1]

        def col(q):   # (B,M) -> (128,1), p = img*64 + g*2 + h
            return np.repeat(q[bs].reshape(2 * M), 2)[:, None].astype(f)

        pr = np.zeros((P, 12), f)
        for qi, q in enumerate((bx1, by1, bx2, by2, gw, gh,
                                (gw * gh + CEPS).astype(f),
                                (bx1 + bx2).astype(f), (by1 + by2).astype(f))):
            pr[:, qi:qi + 1] = col(q)
        pr[:, 9] = (np.arange(P) % 2).astype(f)

        def rect(x):  # (B,M,2,NC1,...) -> (128,NC1,...)
            v = x[bs]
            return v.reshape((P,) + v.shape[3:])

        rb4 = rect(rbox).astype(ml_dtypes.bfloat16)
        gt_tab = np.zeros((P, BPC, 10, M), f)
        for qi, q in enumerate((bx1, by1, bx2, by2,
                                (bx1 + bx2).astype(f), (by1 + by2).astype(f),
                                (gw * gh + CEPS).astype(f), gatan,
                                gmask, lab_all)):
            gt_tab[:, 0, qi, :] = q[bs[0]]
            gt_tab[:, 1, qi, :] = q[bs[1]]

        q_img = np.repeat(np.arange(2), M)
        q_g = np.tile(np.arange(M), 2)
        basehl = np.zeros((64, 6), f)
        i0q = np.zeros((64, 3), f)
        j0q = np.zeros((64, 6), f)
        for li, (off, W, s, R, Rh) in enumerate(LVLS):
            i0v = i0s[li][bs][q_img, q_g]
            j0v = j0s[li][bs][q_img, q_g]
            i0q[:, li] = i0v
            for h in (0, 1):
                basehl[:, h * 3 + li] = q_img * N + off + (j0v + h * Rh) * W + i0v
                j0q[:, h * 3 + li] = j0v + h * Rh

        m = {
            "cls": np.ascontiguousarray(
                cls_logits[bs].astype(ml_dtypes.bfloat16)),
            "tabs": np.ascontiguousarray(gt_tab.reshape(P, BPC * 10 * M)),
            "prcols": pr,
            "rbx1": np.ascontiguousarray(rb4[:, :, 0]),
            "rby1": np.ascontiguousarray(rb4[:, :, 1]),
            "rbx2": np.ascontiguousarray(rb4[:, :, 2]),
            "rby2": np.ascontiguousarray(rb4[:, :, 3]),
            "slog": np.ascontiguousarray(rect(slog).astype(ml_dtypes.bfloat16)),
            "basehl": basehl, "i0q": i0q, "j0q": j0q,
        }
        m.update(hcs)
        maps.append(m)
    return maps


# ===================== device program =====================

def mkap(base_ap, off_elems, dims):
    return AP(tensor=base_ap.tensor, offset=base_ap.offset + off_elems,
              ap=[list(base_ap.ap[0])] + [list(d) for d in dims])


_CUT = 99  # debug cut-points disabled for production

_INPUTS = [
    ("slog", [P, NC1], bf16),
    ("rbx1", [P, NC1], bf16), ("rby1", [P, NC1], bf16),
    ("rbx2", [P, NC1], bf16), ("rby2", [P, NC1], bf16),
    ("tabs", [P, BPC * 10 * M], f32), ("prcols", [P, 12], f32),
    ("basehl", [64, 6], f32), ("i0q", [64, 3], f32), ("j0q", [64, 6], f32),
    ("hc_iotam", [P, 192], f32), ("hc_i16", [P, 384], f32),
    ("hc_i80", [P, 480], f32), ("hc_slotneg", [P, NTILE], f32),
    ("hc_mk", [P, NTILE], f32), ("hc_img6", [P, NTILE], f32),
    ("hc_ident", [P, P], f32), ("hc_iota12", [64, 12], f32),
    ("hc_ones1", [P, 1], bf16),
]


def build_program(dbg=False):
    _patch_act_tables()
    nc = bacc.Bacc("TRN2")
    T = {}
    T["cls"] = nc.dram_tensor("cls", [BPC, N, C], bf16, kind="ExternalInput").ap()
    T["comb"] = nc.dram_tensor("comb", [BPC * N, 148], f32,
                               kind="ExternalInput").ap()
    for k, shape, dty in _INPUTS:
        T[k] = nc.dram_tensor(k, shape, dty, kind="ExternalInput").ap()
    T["out"] = nc.dram_tensor("out", [8, 1], f32, kind="ExternalOutput").ap()
    DBG = {}
    if dbg:
        DBG["d_align2"] = nc.dram_tensor("d_align2", [P, NC1], bf16,
                                         kind="ExternalOutput").ap()
        for name, shape in [
             ("d_ids", [64, 12]), ("d_act", [64, 12]),
            ("d_gid", [P, NTILE]), ("d_ax", [P, NTILE]), ("d_ay", [P, NTILE]),
            ("d_g", [P, NTILE]), ("d_norm", [P, NTILE]),
            ("d_cr", [P, NTILE]), ("d_tss", [P, NTILE]),
            ("d_lc", [P, 8]), ("d_ciou96", [P, 192]),
        ]:
            DBG[name] = nc.dram_tensor(name, shape, f32,
                                       kind="ExternalOutput").ap()

    with ExitStack() as ctx:
        tc = ctx.enter_context(tile.TileContext(nc))
        _body(nc, tc, ctx, T, DBG)
    nc.compile()
    return nc


def _body(nc, tc, ctx, T, DBG):
    pool = ctx.enter_context(tc.tile_pool(name="main", bufs=1))
    tpool = ctx.enter_context(tc.tile_pool(name="tmp", bufs=2))
    spool = ctx.enter_context(tc.tile_pool(name="stream", bufs=3))
    lpool = ctx.enter_context(tc.tile_pool(name="lns", bufs=2))
    pspool = ctx.enter_context(tc.tile_pool(name="ps", bufs=1, space="PSUM"))
    vec, act, gps, pe, syn = nc.vector, nc.scalar, nc.gpsimd, nc.tensor, nc.sync

    def nt(shape, name, dtype=f32):
        return pool.tile(shape, dtype, name=name)

    # ---------- host tensors ----------
    hc = {}
    for name, shape, dty in _INPUTS:
        t = nt(shape, name + "_sb", dty)
        syn.dma_start(t[:], T[name][:])
        hc[name] = t
    ident = hc["hc_ident"]
    prc = hc["prcols"]
    GX1C, GY1C, GX2C, GY2C, GWC, GHC, GAEC, GSXC, GSYC, HC = range(10)

    def pc(i):
        return prc[:, i:i + 1]

    TGX1, TGY1, TGX2, TGY2, TGSX, TGSY, TGAE, TGATAN, TGMASK, TGLAB = range(10)

    def tab(q):
        """[128, 2, 3, 32] AP of GT-table quantity q (img, slot-bcast, m)."""
        return mkap(hc["tabs"][:], q * M, [[10 * M, 2], [0, 3], [1, M]])

    # ================= rect phase (era B) =================
    sl_sig = nt([P, NC1], "sl_sig", bf16)
    act.activation(sl_sig[:], hc["slog"][:], Act.Sigmoid)

    def t1(name, dtype=bf16):
        return nt([P, NC1], name, dtype)

    rbx1, rby1, rbx2, rby2 = (hc[k] for k in ("rbx1", "rby1", "rbx2", "rby2"))
    w2 = t1("w2"); h2 = t1("h2"); pA = t1("pA")
    vec.tensor_tensor(w2[:], rbx2[:], rbx1[:], Alu.subtract)
    vec.tensor_tensor(h2[:], rby2[:], rby1[:], Alu.subtract)
    vec.tensor_tensor(pA[:], w2[:], h2[:], Alu.mult)
    ta = t1("ta"); tb = t1("tb"); tcy = t1("tcy"); inter = t1("inter")
    vec.tensor_scalar(ta[:], rbx2[:], pc(GX2C), None, Alu.min)
    vec.tensor_scalar(tb[:], rbx1[:], pc(GX1C), None, Alu.max)
    vec.tensor_tensor(ta[:], ta[:], tb[:], Alu.subtract)
    vec.tensor_scalar(tcy[:], rby2[:], pc(GY2C), None, Alu.min)
    vec.tensor_scalar(tb[:], rby1[:], pc(GY1C), None, Alu.max)
    vec.tensor_tensor(tcy[:], tcy[:], tb[:], Alu.subtract)
    vec.tensor_scalar(tcy[:], tcy[:], 0.0, None, Alu.max)
    vec.scalar_tensor_tensor(inter[:], ta[:], 0.0, tcy[:], Alu.max, Alu.mult)
    df = t1("df", f32); dr = t1("dr", f32)
    vec.scalar_tensor_tensor(df[:], pA[:], pc(GAEC), inter[:], Alu.add,
                             Alu.subtract)
    vec.reciprocal_approx_fast(dr[:], df[:])
    iou = t1("iou")
    vec.tensor_tensor(iou[:], inter[:], dr[:], Alu.mult)
    vec.tensor_scalar(ta[:], rbx2[:], pc(GX2C), None, Alu.max)
    vec.tensor_scalar(tb[:], rbx1[:], pc(GX1C), None, Alu.min)
    vec.tensor_tensor(ta[:], ta[:], tb[:], Alu.subtract)
    vec.tensor_scalar(tcy[:], rby2[:], pc(GY2C), None, Alu.max)
    vec.tensor_scalar(tb[:], rby1[:], pc(GY1C), None, Alu.min)
    vec.tensor_tensor(tcy[:], tcy[:], tb[:], Alu.subtract)
    c2f = df; c2r = t1("c2r", f32)  # df free between union and den
    vec.tensor_tensor(ta[:], ta[:], ta[:], Alu.mult)
    vec.scalar_tensor_tensor(c2f[:], tcy[:], 1.0, tcy[:], Alu.mult, Alu.mult)
    vec.tensor_tensor(c2f[:], c2f[:], ta[:], Alu.add)
    vec.tensor_scalar(c2f[:], c2f[:], float(CEPS), None, Alu.add)
    vec.reciprocal_approx_fast(c2r[:], c2f[:])
    rho = t1("rho")
    vec.tensor_tensor(ta[:], rbx1[:], rbx2[:], Alu.add)
    vec.tensor_scalar(ta[:], ta[:], pc(GSXC), None, Alu.subtract)
    vec.tensor_tensor(ta[:], ta[:], ta[:], Alu.mult)
    vec.tensor_tensor(tb[:], rby1[:], rby2[:], Alu.add)
    vec.tensor_scalar(tb[:], tb[:], pc(GSYC), None, Alu.subtract)
    vec.tensor_tensor(tb[:], tb[:], tb[:], Alu.mult)
    vec.tensor_tensor(rho[:], ta[:], tb[:], Alu.add)
    # delta-atan identity: atan((w2*gh - gw*h2)/(h2*gh + w2*gw + eps))
    
    vec.tensor_scalar(ta[:], w2[:], pc(GHC), None, Alu.mult)
    vec.tensor_scalar(tb[:], h2[:], pc(GWC), None, Alu.mult)
    vec.tensor_tensor(pA[:], ta[:], tb[:], Alu.subtract)  # num (pA dead)
    vec.tensor_scalar(ta[:], h2[:], pc(GHC), None, Alu.mult)
    vec.tensor_scalar(tb[:], w2[:], pc(GWC), None, Alu.mult)
    vec.tensor_tensor(df[:], ta[:], tb[:], Alu.add)
    vec.tensor_scalar(df[:], df[:], 1e-12, None, Alu.add)
    vec.reciprocal_approx_fast(dr[:], df[:])
    datan = inter  # inter dead after iou
    vec.tensor_tensor(datan[:], pA[:], dr[:], Alu.mult)
    act.activation(datan[:], datan[:], Act.Arctan)            # era B
    v_ = pA  # pA(num) dead after datan
    vec.scalar_tensor_tensor(v_[:], datan[:], float(4.0 / np.pi**2), datan[:],
                             Alu.mult, Alu.mult)
    vec.scalar_tensor_tensor(df[:], iou[:], -1.0, v_[:], Alu.mult, Alu.add)
    vec.tensor_scalar(df[:], df[:], float(1.0 + CEPS), None, Alu.add)
    vec.reciprocal_approx_fast(dr[:], df[:])
    vec.tensor_tensor(ta[:], v_[:], dr[:], Alu.mult)
    vec.tensor_tensor(ta[:], v_[:], ta[:], Alu.mult)          # v*alpha
    cio = t1("cio")
    vec.tensor_tensor(tb[:], rho[:], c2r[:], Alu.mult)
    vec.scalar_tensor_tensor(cio[:], tb[:], -0.25, iou[:], Alu.mult, Alu.add)
    vec.tensor_tensor(cio[:], cio[:], ta[:], Alu.subtract)
    # align^2 = sig * ov^12 * maskc
    o2 = w2  # w2/h2 dead after den
    vec.scalar_tensor_tensor(o2[:], cio[:], 0.0, cio[:], Alu.max, Alu.mult)
    vec.tensor_tensor(ta[:], o2[:], o2[:], Alu.mult)
    vec.tensor_tensor(tb[:], ta[:], ta[:], Alu.mult)
    vec.tensor_tensor(ta[:], tb[:], ta[:], Alu.mult)          # ov^12
    al2 = iou  # iou dead after cio
    vec.tensor_tensor(al2[:], ta[:], sl_sig[:], Alu.mult)
    if DBG:
        syn.dma_start(DBG["d_align2"][:], al2[:])

    # ================= ERA B stream: sig = Sigmoid(-x) =================
    sigbuf = nt([P, NT * STREAM_F], "sigbuf", bf16)
    cls_flat = T["cls"].rearrange("b n c -> (b n c)")
    for it in range(NT):
        x = spool.tile([P, STREAM_F], bf16, tag="sx")
        src = AP(tensor=cls_flat.tensor, offset=it * P * STREAM_F,
                 ap=[[STREAM_F, P], [1, STREAM_F]])
        syn.dma_start(x[:], src)
        act.activation(sigbuf[:, it * STREAM_F:(it + 1) * STREAM_F], x[:],
                       Act.Sigmoid, scale=-1.0)

    def finalize_early(acc):
        LCx = nt([P, 8], "LCx")
        vec.memset(LCx[:], 0.0)
        vec.tensor_reduce(LCx[:, 0:1], acc[:], mybir.AxisListType.X, Alu.add)
        psx = pspool.tile([8, P], f32, tag="psF", space="PSUM")
        pe.transpose(psx[:], LCx[:], ident[:])
        osb = nt([8, 1], "osb")
        vec.tensor_reduce(osb[:], psx[:], mybir.AxisListType.X, Alu.add)
        syn.dma_start(T["out"][:], osb[:])

    # ================= top-16 per half + merge =================
    vals16 = nt([P, 16], "vals16", bf16)
    idx16 = nt([P, 16], "idx16", dt.uint32)
    amr = rho  # rho dead after cio
    vec.max(vals16[:, 0:8], al2[:])
    vec.max_index(idx16[:, 0:8], vals16[:, 0:8], al2[:])
    vec.match_replace(amr[:], vals16[:, 0:8], al2[:], float(NEG))
    vec.max(vals16[:, 8:16], amr[:])
    vec.max_index(idx16[:, 8:16], vals16[:, 8:16], amr[:])
    idx16f = nt([P, 16], "idx16f")
    vec.tensor_copy(idx16f[:], idx16[:])
    vec.scalar_tensor_tensor(idx16f[:], pc(HC).to_broadcast([P, 16]),
                             float(NC1), idx16f[:], Alu.mult, Alu.add)
    v32 = nt([64, 32], "v32", bf16)
    p32 = nt([64, 32], "p32")
    syn.dma_start(v32[:], vals16[:])
    syn.dma_start(p32[:], idx16f[:])
    mm = nt([64, 16], "mm", bf16)
    v32r = nt([64, 32], "v32r", bf16)
    vec.max(mm[:, 0:8], v32[:])
    vec.match_replace(v32r[:], mm[:, 0:8], v32[:], float(NEG))
    vec.max(mm[:, 8:16], v32r[:])
    thr = nt([64, 1], "thr")
    vec.tensor_copy(thr[:], mm[:, 9:10])
    vec.tensor_scalar(thr[:], thr[:], float(EPS2), None, Alu.max)
    Fl = nt([64, 32], "Fl")
    ta32 = nt([64, 32], "ta32")
    vec.tensor_scalar(Fl[:], v32[:], thr[:], None, Alu.is_ge)
    vec.tensor_scalar(ta32[:], v32[:], float(EPS2), None, Alu.is_gt)
    vec.tensor_tensor(Fl[:], Fl[:], ta32[:], Alu.mult)
    rank = nt([64, 32], "rank")
    vec.tensor_tensor_scan(rank[:], Fl[:], Fl[:], 0.0, Alu.add, Alu.bypass)
    fid = nt([64, 32], "fid")
    vec.tensor_tensor(fid[:], Fl[:], p32[:], Alu.mult)
    ids10 = nt([64, 12], "ids10")
    act10 = nt([64, 12], "act10")
    vec.memset(ids10[:], 0.0)
    for k in range(K10):
        vec.scalar_tensor_tensor(ta32[:], rank[:], float(k + 1), fid[:],
                                 Alu.is_equal, Alu.mult,
                                 accum_out=ids10[:, k:k + 1])
    cnt10 = nt([64, 1], "cnt10")
    vec.tensor_reduce(cnt10[:], Fl[:], mybir.AxisListType.X, Alu.add)
    vec.tensor_scalar(cnt10[:], cnt10[:], 10.0, None, Alu.min)
    vec.tensor_scalar(act10[:], hc["hc_iota12"][:], cnt10[:], None, Alu.is_lt)
    if DBG:
        syn.dma_start(DBG["d_ids"][:], ids10[:])
        syn.dma_start(DBG["d_act"][:], act10[:])

    # ================= decode ext ids =================
    dc = nt([64, 12 * 12], "dcode")

    def dcc(i):
        return dc[:, i * 12:(i + 1) * 12]

    HF, POS, L1, L2, POSL, RR, CC_, WL_, GID, AXQ, AYQ, SL_ = range(12)
    vec.tensor_scalar(dcc(HF), ids10[:], float(NC1), None, Alu.is_ge)
    vec.scalar_tensor_tensor(dcc(POS), dcc(HF), float(-NC1), ids10[:],
                             Alu.mult, Alu.add)
    vec.tensor_scalar(dcc(L1), dcc(POS), float(SEGOFF[1]), None, Alu.is_ge)
    vec.tensor_scalar(dcc(L2), dcc(POS), float(SEGOFF[2]), None, Alu.is_ge)
    vec.scalar_tensor_tensor(dcc(POSL), dcc(L1), float(-SEGOFF[1]), dcc(POS),
                             Alu.mult, Alu.add)
    vec.scalar_tensor_tensor(dcc(POSL), dcc(L2), float(SEGOFF[1] - SEGOFF[2]),
                             dcc(POSL), Alu.mult, Alu.add)
    rinv = nt([64, 12], "rinv")
    vec.tensor_scalar(rinv[:], dcc(L1), float(1 / 22 - 1 / 42), None, Alu.mult)
    vec.scalar_tensor_tensor(rinv[:], dcc(L2), float(1 / 12 - 1 / 22),
                             rinv[:], Alu.mult, Alu.add)
    vec.tensor_scalar(rinv[:], rinv[:], float(1 / 42), None, Alu.add)
    ri = nt([64, 12], "ri", dt.int32)
    vec.tensor_tensor(dcc(RR), dcc(POSL), rinv[:], Alu.mult)
    vec.tensor_scalar(dcc(RR), dcc(RR), float(1e-4 - 0.4999), None, Alu.add)
    vec.tensor_copy(ri[:], dcc(RR))    # int cast rounds-to-nearest -> floor
    vec.tensor_copy(dcc(RR), ri[:])
    rl = nt([64, 12], "rl")
    vec.tensor_scalar(rl[:], dcc(L1), -20.0, None, Alu.mult)
    vec.scalar_tensor_tensor(rl[:], dcc(L2), -10.0, rl[:], Alu.mult, Alu.add)
    vec.tensor_scalar(rl[:], rl[:], 42.0, None, Alu.add)
    vec.tensor_tensor(rl[:], dcc(RR), rl[:], Alu.mult)
    vec.tensor_tensor(dcc(CC_), dcc(POSL), rl[:], Alu.subtract)
    vec.tensor_scalar(dcc(WL_), dcc(L1), -80.0, None, Alu.mult)
    vec.scalar_tensor_tensor(dcc(WL_), dcc(L2), -40.0, dcc(WL_), Alu.mult, Alu.add)
    vec.tensor_scalar(dcc(WL_), dcc(WL_), 160.0, None, Alu.add)
    lh0 = nt([64, 12], "lh0"); lh1 = nt([64, 12], "lh1"); lh2 = nt([64, 12], "lh2")
    vec.tensor_copy(lh2[:], dcc(L2))
    vec.tensor_tensor(lh1[:], dcc(L1), dcc(L2), Alu.subtract)
    vec.tensor_scalar(lh0[:], dcc(L1), -1.0, 1.0, Alu.mult, Alu.add)
    seltmp = nt([64, 12], "seltmp")

    def sel3(dst, tbl, coff):
        vec.tensor_scalar(dst, lh0[:], tbl[:, coff:coff + 1], None, Alu.mult)
        vec.tensor_scalar(seltmp[:], lh1[:], tbl[:, coff + 1:coff + 2],
                          None, Alu.mult)
        vec.tensor_tensor(dst, dst, seltmp[:], Alu.add)
        vec.tensor_scalar(seltmp[:], lh2[:], tbl[:, coff + 2:coff + 3],
                          None, Alu.mult)
        vec.tensor_tensor(dst, dst, seltmp[:], Alu.add)

    b_h0 = nt([64, 12], "b_h0"); b_h1 = nt([64, 12], "b_h1")
    sel3(b_h0[:], hc["basehl"], 0)
    sel3(b_h1[:], hc["basehl"], 3)
    vec.tensor_tensor(b_h1[:], b_h1[:], b_h0[:], Alu.subtract)
    vec.tensor_tensor(b_h1[:], b_h1[:], dcc(HF), Alu.mult)
    vec.tensor_tensor(b_h0[:], b_h0[:], b_h1[:], Alu.add)     # base(q,h,l)
    vec.tensor_tensor(dcc(GID), dcc(RR), dcc(WL_), Alu.mult)
    vec.tensor_tensor(dcc(GID), dcc(GID), b_h0[:], Alu.add)
    vec.tensor_tensor(dcc(GID), dcc(GID), dcc(CC_), Alu.add)
    vec.tensor_scalar(dcc(SL_), dcc(L1), 8.0, None, Alu.mult)
    vec.scalar_tensor_tensor(dcc(SL_), dcc(L2), 16.0, dcc(SL_), Alu.mult, Alu.add)
    vec.tensor_scalar(dcc(SL_), dcc(SL_), 8.0, None, Alu.add)
    i0sel = nt([64, 12], "i0sel")
    sel3(i0sel[:], hc["i0q"], 0)
    j_h0 = nt([64, 12], "j_h0"); j_h1 = nt([64, 12], "j_h1")
    sel3(j_h0[:], hc["j0q"], 0)
    sel3(j_h1[:], hc["j0q"], 3)
    vec.tensor_tensor(j_h1[:], j_h1[:], j_h0[:], Alu.subtract)
    vec.tensor_tensor(j_h1[:], j_h1[:], dcc(HF), Alu.mult)
    vec.tensor_tensor(j_h0[:], j_h0[:], j_h1[:], Alu.add)     # j0h(q,h,l)
    vec.tensor_tensor(dcc(AXQ), i0sel[:], dcc(CC_), Alu.add)
    vec.tensor_scalar(dcc(AXQ), dcc(AXQ), 0.5, None, Alu.add)
    vec.tensor_tensor(dcc(AXQ), dcc(AXQ), dcc(SL_), Alu.mult)
    vec.tensor_tensor(dcc(AYQ), j_h0[:], dcc(RR), Alu.add)
    vec.tensor_scalar(dcc(AYQ), dcc(AYQ), 0.5, None, Alu.add)
    vec.tensor_tensor(dcc(AYQ), dcc(AYQ), dcc(SL_), Alu.mult)

    # ================= move [64,12] -> [128,6] =================
    sid6 = nt([P, NTILE], "sid6")
    sact6 = nt([P, NTILE], "sact6")
    ax6 = nt([P, NTILE], "ax6")
    ay6 = nt([P, NTILE], "ay6")
    for img in range(BPC):
        rsl = slice(img * 32, img * 32 + 32)
        csl = slice(3 * img, 3 * img + 3)
        syn.dma_start(sid6[:, csl], dc[rsl, GID * 12:GID * 12 + 12])
        syn.dma_start(sact6[:, csl], act10[rsl, :])
        syn.dma_start(ax6[:, csl], dc[rsl, AXQ * 12:AXQ * 12 + 12])
        syn.dma_start(ay6[:, csl], dc[rsl, AYQ * 12:AYQ * 12 + 12])
    if DBG:
        syn.dma_start(DBG["d_gid"][:], sid6[:])
        syn.dma_start(DBG["d_ax"][:], ax6[:])
        syn.dma_start(DBG["d_ay"][:], ay6[:])

    sid_i = nt([P, NTILE], "sid_i", dt.int32)
    vec.tensor_copy(sid_i[:], sid6[:])
    nloc6 = nt([P, NTILE], "nloc6")
    vec.scalar_tensor_tensor(nloc6[:], hc["hc_img6"][:], float(-N), sid6[:],
                             Alu.mult, Alu.add)
    rstp6 = nt([P, NTILE], "rstp6")
    t6a = nt([P, NTILE], "t6a"); t6b = nt([P, NTILE], "t6b")
    vec.tensor_scalar(t6a[:], nloc6[:], 25600.0, None, Alu.is_ge)
    vec.tensor_scalar(t6b[:], nloc6[:], 32000.0, None, Alu.is_ge)
    vec.tensor_scalar(t6a[:], t6a[:], 8.0, 8.0, Alu.mult, Alu.add)
    vec.scalar_tensor_tensor(rstp6[:], t6b[:], 16.0, t6a[:], Alu.mult, Alu.add)
    vec.reciprocal(rstp6[:], rstp6[:])

    # ---- slot gathers: one combined row (dist|boxd|cls) per slot ----
    crow = nt([P, NTILE, 148], "crow")
    for t in range(NTILE):
        io = bass.IndirectOffsetOnAxis(ap=sid_i[:, t:t + 1], axis=0)
        gps.indirect_dma_start(out=crow[:, t, :], out_offset=None,
                               in_=T["comb"], in_offset=io)

    def distrow_ap(dims):
        return mkap(crow[:], 0, dims)

    def pbox_col(i):
        return mkap(crow[:], 64 + i, [[148, NTILE]])

    def clsrow_ap():
        return mkap(crow[:], 68, [[148, NTILE], [1, 80]])

    if _CUT <= 4:
        bg0 = nt([P, 4], "bg0")
        vec.memset(bg0[:], 0.0)
        finalize_early(bg0)
        return

    # ---- era gate columns ----
    zcol = nt([P, 1], "zcol")
    ocol = nt([P, 1], "ocol")
    vec.tensor_scalar(zcol[:], sigbuf[:, NT * STREAM_F - 1:NT * STREAM_F],
                      0.0, None, Alu.mult)
    vec.tensor_scalar(ocol[:], zcol[:], 1.0, None, Alu.add)

    # ================= ERA A stream (issued in groups, interleaved with
    # the phase-2 front so each engine queue can fill its stall gaps) ======
    # scalar: L=Ln(sig), s2=Square(1-sig); vector: T=s2*L; PE: ones^T @ T
    # accumulated into one PSUM bank.  bg = -0.75 * sum(T).
    psbg = pspool.tile([1, 512], f32, tag="psbg", space="PSUM")
    CHUNKS = [(c, min(c + 512, STREAM_F)) for c in range(0, STREAM_F, 512)]

    def emit_stream(lo, hi):
        for it in range(lo, hi):
            sl = slice(it * STREAM_F, (it + 1) * STREAM_F)
            L = lpool.tile([P, STREAM_F], bf16, tag="L")
            act.activation(L[:], sigbuf[:, sl], Act.Ln, bias=zcol[:])
            s2 = lpool.tile([P, STREAM_F], bf16, tag="s2")
            act.activation(s2[:], sigbuf[:, sl], Act.Square, scale=-1.0,
                           bias=ocol[:])
            vec.tensor_tensor(s2[:], s2[:], L[:], Alu.mult)
            for ci, (c0, c1) in enumerate(CHUNKS):
                pe.matmul(psbg[:, 0:c1 - c0], hc["hc_ones1"][:], s2[:, c0:c1],
                          start=(it == 0 and ci == 0),
                          stop=(it == NT - 1 and ci == len(CHUNKS) - 1))

    emit_stream(0, 6)
    if _CUT <= 5:
        emit_stream(6, NT)
        bg0 = nt([P, 4], "bg05")
        vec.memset(bg0[:], 0.0)
        finalize_early(bg0)
        return

    # ================= pq + ciou96 (fused imgs) =================
    pq = nt([P, 12 * NTILE], "pq")

    def pqc(i):
        return pq[:, i * NTILE:(i + 1) * NTILE]

    PX1, PY1, PX2, PY2, PW, PH, PAE, PSX, PSY, PATAN, QAX, QAY = range(12)
    for i, src_i in [(PX1, 0), (PY1, 1), (PX2, 2), (PY2, 3)]:
        vec.tensor_copy(pqc(i), pbox_col(src_i))
    vec.tensor_copy(pqc(QAX), ax6[:])
    vec.tensor_copy(pqc(QAY), ay6[:])
    vec.tensor_tensor(pqc(PW), pqc(PX2), pqc(PX1), Alu.subtract)
    vec.tensor_tensor(pqc(PH), pqc(PY2), pqc(PY1), Alu.subtract)
    vec.tensor_tensor(pqc(PAE), pqc(PW), pqc(PH), Alu.mult)
    vec.tensor_scalar(pqc(PAE), pqc(PAE), float(CEPS), None, Alu.add)
    vec.tensor_tensor(pqc(PSX), pqc(PX1), pqc(PX2), Alu.add)
    vec.tensor_tensor(pqc(PSY), pqc(PY1), pqc(PY2), Alu.add)
    vec.tensor_scalar(t6a[:], pqc(PH), float(CEPS), None, Alu.add)
    vec.reciprocal(t6a[:], t6a[:])
    vec.tensor_tensor(t6a[:], pqc(PW), t6a[:], Alu.mult)
    act.activation(pqc(PATAN), t6a[:], Act.Arctan)            # era B

    def colx2(t6ap, inner=M):
        return mkap(t6ap, 0, [[3, 2], [1, 3], [0, inner]])

    u1 = nt([P, 192], "u1"); u2 = nt([P, 192], "u2")
    u3 = nt([P, 192], "u3"); u4 = nt([P, 192], "u4")
    ciou96 = nt([P, 192], "ciou96")
    ov96 = nt([P, 192], "ov96")
    vec.tensor_tensor(u1[:], colx2(pqc(PX2)), tab(TGX2), Alu.min)
    vec.tensor_tensor(u2[:], colx2(pqc(PX1)), tab(TGX1), Alu.max)
    vec.tensor_tensor(u1[:], u1[:], u2[:], Alu.subtract)
    vec.tensor_tensor(u3[:], colx2(pqc(PY2)), tab(TGY2), Alu.min)
    vec.tensor_tensor(u2[:], colx2(pqc(PY1)), tab(TGY1), Alu.max)
    vec.tensor_tensor(u3[:], u3[:], u2[:], Alu.subtract)
    vec.tensor_scalar(u3[:], u3[:], 0.0, None, Alu.max)
    vec.scalar_tensor_tensor(u1[:], u1[:], 0.0, u3[:], Alu.max, Alu.mult)
    vec.tensor_tensor(u2[:], colx2(pqc(PAE)), tab(TGAE), Alu.add)
    vec.tensor_tensor(u2[:], u2[:], u1[:], Alu.subtract)
    vec.reciprocal_approx_fast(u2[:], u2[:])
    vec.tensor_tensor(u2[:], u1[:], u2[:], Alu.mult)                  # iou
    vec.tensor_tensor(u1[:], colx2(pqc(PX2)), tab(TGX2), Alu.max)
    vec.tensor_tensor(u3[:], colx2(pqc(PX1)), tab(TGX1), Alu.min)
    vec.tensor_tensor(u1[:], u1[:], u3[:], Alu.subtract)
    vec.tensor_tensor(u1[:], u1[:], u1[:], Alu.mult)
    vec.tensor_tensor(u3[:], colx2(pqc(PY2)), tab(TGY2), Alu.max)
    vec.tensor_tensor(u4[:], colx2(pqc(PY1)), tab(TGY1), Alu.min)
    vec.tensor_tensor(u3[:], u3[:], u4[:], Alu.subtract)
    vec.tensor_tensor(u3[:], u3[:], u3[:], Alu.mult)
    vec.tensor_tensor(u3[:], u1[:], u3[:], Alu.add)
    vec.tensor_scalar(u3[:], u3[:], float(CEPS), None, Alu.add)
    vec.reciprocal_approx_fast(u3[:], u3[:])
    vec.tensor_tensor(u1[:], colx2(pqc(PSX)), tab(TGSX), Alu.subtract)
    vec.tensor_tensor(u1[:], u1[:], u1[:], Alu.mult)
    vec.tensor_tensor(u4[:], colx2(pqc(PSY)), tab(TGSY), Alu.subtract)
    vec.tensor_tensor(u4[:], u4[:], u4[:], Alu.mult)
    vec.tensor_tensor(u1[:], u1[:], u4[:], Alu.add)
    vec.tensor_tensor(u1[:], u1[:], u3[:], Alu.mult)
    vec.scalar_tensor_tensor(u1[:], u1[:], -0.25, u2[:], Alu.mult, Alu.add)
    vec.tensor_tensor(u3[:], colx2(pqc(PATAN)), tab(TGATAN), Alu.subtract)
    vec.tensor_tensor(u3[:], u3[:], u3[:], Alu.mult)
    vec.tensor_scalar(u3[:], u3[:], float(4.0 / np.pi**2), None, Alu.mult)
    vec.scalar_tensor_tensor(u4[:], u2[:], -1.0, u3[:], Alu.mult, Alu.add)
    vec.tensor_scalar(u4[:], u4[:], float(1.0 + CEPS), None, Alu.add)
    vec.reciprocal_approx_fast(u4[:], u4[:])
    vec.tensor_tensor(u3[:], u3[:], u3[:], Alu.mult)
    vec.tensor_tensor(u4[:], u3[:], u4[:], Alu.mult)
    vec.tensor_tensor(ciou96[:], u1[:], u4[:], Alu.subtract)
    vec.tensor_scalar(ov96[:], ciou96[:], 0.0, None, Alu.max)
    if DBG:
        syn.dma_start(DBG["d_ciou96"][:], ciou96[:])

    # ---- best + dedup + g6 ----
    best6 = nt([P, NTILE], "best6")
    bm = nt([P, 8], "bmax")
    bi = nt([P, 8], "bidx", dt.uint32)
    for t in range(NTILE):
        img, u = t // 3, t % 3
        blk = ov96[:, (img * 3 + u) * 32:(img * 3 + u) * 32 + 32]
        vec.max(bm[:], blk)
        vec.max_index(bi[:], bm[:], blk)
        vec.tensor_copy(best6[:, t:t + 1], bi[:, 0:1])
    key6 = nt([P, NTILE], "key6")
    mski = nt([P, NTILE], "mski", dt.int32)
    vec.tensor_copy(mski[:], sact6[:])
    vec.select(key6[:], mski[:], sid6[:], hc["hc_slotneg"][:])
    keyb = nt([P, NTILE * P], "keyb")
    for t in range(NTILE):
        ps = pspool.tile([P, P], f32, tag="tps", space="PSUM")
        pe.transpose(ps[:], key6[:, t:t + 1].to_broadcast([P, P]), ident[:])
        act.activation(keyb[:, t * P:(t + 1) * P], ps[:], Act.Copy)
    cnt6 = nt([P, NTILE], "cnt6")
    zz6 = nt([P, NTILE * P], "zz6", bf16)
    for t in range(NTILE):
        vec.tensor_scalar(zz6[:], keyb[:], key6[:, t:t + 1], None, Alu.is_equal,
                          Alu.add, accum_out=cnt6[:, t:t + 1])
    scale6 = nt([P, NTILE], "scale6")
    vec.tensor_scalar(scale6[:], cnt6[:], 1.0, None, Alu.max)
    vec.reciprocal(scale6[:], scale6[:])
    vec.tensor_tensor(scale6[:], scale6[:], sact6[:], Alu.mult)
    multi6 = nt([P, NTILE], "multi6")
    vec.scalar_tensor_tensor(multi6[:], cnt6[:], 1.0, sact6[:], Alu.is_gt, Alu.mult)
    g6 = nt([P, NTILE], "g6")
    vec.tensor_copy(mski[:], multi6[:])
    vec.select(g6[:], mski[:], best6[:], hc["hc_mk"][:])
    if DBG:
        syn.dma_start(DBG["d_g"][:], g6[:])

    G96 = nt([P, 192], "G96")
    vec.tensor_tensor(G96[:], hc["hc_iotam"][:], colx2(g6[:]), Alu.is_equal)
    vec.tensor_tensor(G96[:], G96[:], colx2(sact6[:]), Alu.mult)

    # ---- extracts ----
    ext = nt([P, 10 * NTILE], "ext")

    def ec(i):
        return ext[:, i * NTILE:(i + 1) * NTILE]

    E_CR, E_OV, E_LAB, E_TX1, E_TY1, E_TX2, E_TY2, E_GM, E_RAT, E_NRM = range(10)
    prod = nt([P, 192], "prod")

    def extract(dst6, srcap):
        vec.tensor_tensor(prod[:], srcap, G96[:], Alu.mult)
        vec.tensor_reduce(dst6, prod[:].rearrange("p (t m) -> p t m", m=M),
                          mybir.AxisListType.X, Alu.add)

    extract(ec(E_CR), ciou96[:])
    extract(ec(E_OV), ov96[:])
    extract(ec(E_LAB), tab(TGLAB))
    extract(ec(E_TX1), tab(TGX1))
    extract(ec(E_TY1), tab(TGY1))
    extract(ec(E_TX2), tab(TGX2))
    extract(ec(E_TY2), tab(TGY2))
    extract(ec(E_GM), tab(TGMASK))

    # ---- xk + era-B tail sigmoid ----
    oh8 = nt([P, NTILE * 80], "oh8")
    vec.tensor_tensor(oh8[:], hc["hc_i80"][:],
                      mkap(ec(E_LAB), 0, [[1, NTILE], [0, 80]]), Alu.is_equal)
    vec.tensor_tensor(oh8[:], oh8[:], clsrow_ap(), Alu.mult)
    xk = nt([P, NTILE], "xk")
    vec.tensor_reduce(xk[:], oh8[:].rearrange("p (t c) -> p t c", c=80),
                      mybir.AxisListType.X, Alu.add)
    sigkn = nt([P, NTILE], "sigkn")
    act.activation(sigkn[:], xk[:], Act.Sigmoid, scale=-1.0)  # era B

    # ---- per-slot era-A pieces ----
    Lk = nt([P, NTILE], "Lk")
    act.activation(Lk[:], sigkn[:], Act.Ln, bias=zcol[:])
    ln1m = nt([P, NTILE], "ln1m")
    act.activation(ln1m[:], sigkn[:], Act.Ln, scale=-1.0, bias=ocol[:])
    rsk = nt([P, NTILE], "rsk")
    act.activation(rsk[:], ln1m[:], Act.Exp, scale=0.5, bias=zcol[:])
    sigk = nt([P, NTILE], "sigk")
    vec.tensor_scalar(sigk[:], sigkn[:], -1.0, 1.0, Alu.mult, Alu.add)

    al_s = nt([P, NTILE], "al_s")
    vec.tensor_tensor(t6a[:], ec(E_OV), ec(E_OV), Alu.mult)
    vec.tensor_tensor(t6b[:], t6a[:], t6a[:], Alu.mult)
    vec.tensor_tensor(t6b[:], t6b[:], t6a[:], Alu.mult)       # ov^6
    vec.tensor_tensor(al_s[:], t6b[:], rsk[:], Alu.mult)
    vec.tensor_tensor(al_s[:], al_s[:], ec(E_GM), Alu.mult)
    # in-box mask at (slot, g6): reassigned anchors may sit outside best's box
    m1 = nt([P, NTILE], "m1s"); m2 = nt([P, NTILE], "m2s")
    vec.tensor_tensor(m1[:], pqc(QAX), ec(E_TX1), Alu.subtract)
    vec.tensor_tensor(m2[:], pqc(QAY), ec(E_TY1), Alu.subtract)
    vec.tensor_tensor(m1[:], m1[:], m2[:], Alu.min)
    vec.tensor_scalar(m1[:], m1[:], float(EPS), None, Alu.is_gt)
    vec.tensor_tensor(m2[:], pqc(QAX), ec(E_TX2), Alu.subtract)
    vec.tensor_tensor(al_s[:], al_s[:], m1[:], Alu.mult)
    vec.tensor_tensor(m1[:], pqc(QAY), ec(E_TY2), Alu.subtract)
    vec.tensor_tensor(m2[:], m2[:], m1[:], Alu.max)
    vec.tensor_scalar(m2[:], m2[:], float(-EPS), None, Alu.is_lt)
    vec.tensor_tensor(al_s[:], al_s[:], m2[:], Alu.mult)

    A192 = nt([P, 192], "A192")
    O192 = nt([P, 192], "O192")
    vec.tensor_tensor(A192[:], G96[:], colx2(al_s[:]), Alu.mult)
    vec.tensor_tensor(O192[:], G96[:], colx2(ec(E_OV)), Alu.mult)
    pmA = nt([P, 64], "pmA")
    pmO = nt([P, 64], "pmO")
    for (srcb, dstb) in ((A192, pmA), (O192, pmO)):
        s3 = srcb[:].rearrange("p (i t m) -> p i t m", i=2, t=3)
        d2 = dstb[:].rearrange("p (i m) -> p i m", i=2)
        vec.tensor_tensor(d2, s3[:, :, 0, :], s3[:, :, 1, :], Alu.max)
        vec.tensor_tensor(d2, d2, s3[:, :, 2, :], Alu.max)
    psA = pspool.tile([64, P], f32, tag="psA", space="PSUM")
    pe.transpose(psA[:], pmA[:], ident[:])
    psO = pspool.tile([64, P], f32, tag="psO", space="PSUM")
    pe.transpose(psO[:], pmO[:], ident[:])
    posag = nt([64, 4], "posag")
    vec.tensor_reduce(posag[:, 0:1], psA[:], mybir.AxisListType.X, Alu.max)
    vec.tensor_reduce(posag[:, 1:2], psO[:], mybir.AxisListType.X, Alu.max)
    vec.tensor_scalar(posag[:, 0:1], posag[:, 0:1], float(EPS), None, Alu.add)
    vec.reciprocal(posag[:, 2:3], posag[:, 0:1])
    vec.tensor_tensor(posag[:, 3:4], posag[:, 1:2], posag[:, 2:3], Alu.mult)
    psR = pspool.tile([P, 64], f32, tag="psR", space="PSUM")
    pe.transpose(psR[:], posag[:, 3:4].to_broadcast([64, P]), ident[0:64, 0:64])
    rat_tab = nt([P, 64], "rat_tab")
    act.activation(rat_tab[:], psR[:], Act.Copy)
    extract(ec(E_RAT), mkap(rat_tab[:], 0, [[M, 2], [0, 3], [1, M]]))
    vec.tensor_tensor(ec(E_NRM), al_s[:], ec(E_RAT), Alu.mult)
    emit_stream(6, 12)
    if DBG:
        syn.dma_start(DBG["d_norm"][:], ec(E_NRM))
        syn.dma_start(DBG["d_cr"][:], ec(E_CR))

    # ---- losses ----
    LC = nt([P, 8], "LC")
    vec.memset(LC[:], 0.0)
    tssc = nt([P, NTILE], "tssc")
    vec.tensor_tensor(tssc[:], scale6[:], ec(E_NRM), Alu.mult)
    vec.tensor_reduce(LC[:, 2:3], tssc[:], mybir.AxisListType.X, Alu.add)
    w6 = nt([P, NTILE], "w6")
    vec.tensor_scalar(w6[:], ec(E_CR), -1.0, 1.0, Alu.mult, Alu.add)
